# revision 10
# baseline (speedup 1.0000x reference)
# Trainium2 Bass kernel for nn_BDH_66056597013022 (dense_transformer).
#
# Model (per reference):
#   v = LN(emb_w[tokens])                                  [B,T,D]
#   6x: x  = relu(v @ Dx_h)            per head            [B,H,T,Dh]
#       xr = RoPE(x)
#       S  = xr @ xr^T                 (no softmax)        [B,H,T,T]
#       a  = S @ v                                         [B,H,T,D]
#       y  = relu(a @ Dy_h) * x                            [B,H,T,Dh]
#       v  = LN(v + LN(concat_h(y) @ E))
#   out = v @ readout                                      [B,T,V]
#
# Shapes: B=4 T=1024 H=4 N=4096 D=256 L=6 V=256, Dh=N/H=1024.
#
# Sharding (8 cores): core c -> batch b=c//2, head-pair hp=c%2 (heads 2hp,2hp+1).
# All per-head work is local; the only cross-core coupling is the head-sum in
# z = y @ E, handled with a 2-rank AllReduce per layer between cores {2b,2b+1}.
# Both cores of a pair then redundantly compute the LN/v-update, so the whole
# forward stays on-device; even cores' outputs are returned.
#
# On-chip layouts per core (SBUF):
#   v    [T,D]   8 tiles [128,256]   (token rows on partitions)
#   vT   [D,T]   2 tiles [128,1024]  (for contractions over D)
#   xT,xrT,yT [Dh,T] 8 tiles [128,1024] each head (Dh on partitions)
#   S streamed per 128-row block [128,1024]; aT [D,T] 2 tiles.
# All matmuls are out = lhsT.T @ rhs with K<=128 on partitions; S is
# numerically symmetric so its [t,s] tiles serve as [s,t] operands directly.

import os
import numpy as np

B, T, H, N, D, L, V = 4, 1024, 4, 4096, 256, 6, 256
Dh = N // H
EPS = 1e-5
NCORES = 8
P = 128
NT = T // P  # 8 tiles of tokens
ND = D // P  # 2 tiles of model dim
NDh = Dh // P  # 8 tiles of head dim

_CACHE = {}
LAST_RESULT = None


def _build_program():
    from contextlib import ExitStack

    import concourse.bass as bass
    import concourse.bacc as bacc
    import concourse.tile as tile
    import concourse.mybir as mybir
    from concourse.masks import make_identity

    f32 = mybir.dt.float32
    AF = mybir.ActivationFunctionType
    ALU = mybir.AluOpType
    ts = bass.ts

    nc = bacc.Bacc("TRN2", target_bir_lowering=False, debug=False,
                   enable_asserts=False, num_devices=NCORES)

    d_oh = nc.dram_tensor("onehotT", [V, T], f32, kind="ExternalInput").ap()
    d_ew = nc.dram_tensor("emb_w", [V, D], f32, kind="ExternalInput").ap()
    d_dx = nc.dram_tensor("dx", [2 * D, Dh], f32, kind="ExternalInput").ap()
    d_dy = nc.dram_tensor("dy", [2 * D, Dh], f32, kind="ExternalInput").ap()
    d_eh = nc.dram_tensor("eh", [2 * Dh, D], f32, kind="ExternalInput").ap()
    d_cos = nc.dram_tensor("cosT", [Dh // 2, T], f32, kind="ExternalInput").ap()
    d_sin = nc.dram_tensor("sinT", [Dh // 2, T], f32, kind="ExternalInput").ap()
    d_ro = nc.dram_tensor("readout", [D, V], f32, kind="ExternalInput").ap()
    d_out = nc.dram_tensor("out", [T, V], f32, kind="ExternalOutput").ap()

    with tile.TileContext(nc) as tc, ExitStack() as ctx:
        wpool = ctx.enter_context(tc.tile_pool(name="weights", bufs=1))
        vpool = ctx.enter_context(tc.tile_pool(name="vpool", bufs=1))
        xpool = ctx.enter_context(tc.tile_pool(name="xpool", bufs=4))
        xrpool = ctx.enter_context(tc.tile_pool(name="xrpool", bufs=8))
        spool = ctx.enter_context(tc.tile_pool(name="spool", bufs=2))
        apool = ctx.enter_context(tc.tile_pool(name="apool", bufs=2))
        ypool = ctx.enter_context(tc.tile_pool(name="ypool", bufs=3))
        zpool = ctx.enter_context(tc.tile_pool(name="zpool", bufs=1))
        lnpool = ctx.enter_context(tc.tile_pool(name="lnpool", bufs=3))
        stpool = ctx.enter_context(tc.tile_pool(name="stpool", bufs=4))
        rtpool = ctx.enter_context(tc.tile_pool(name="rtpool", bufs=2))
        psA = ctx.enter_context(tc.tile_pool(name="psA", bufs=2, space="PSUM"))
        psB = ctx.enter_context(tc.tile_pool(name="psB", bufs=2, space="PSUM"))
        dpool = ctx.enter_context(tc.tile_pool(name="drampool", bufs=2, space="DRAM"))

        # ---- persistent weights ----
        dx_sb = []
        dy_sb = []
        for i in range(4):
            dxt = wpool.tile([P, Dh], f32, tag=f"dx{i}", name=f"dx{i}")
            nc.sync.dma_start(dxt[:], d_dx[ts(i, P), :])
            dx_sb.append(dxt)
        for i in range(4):
            dyt = wpool.tile([P, Dh], f32, tag=f"dy{i}", name=f"dy{i}")
            nc.sync.dma_start(dyt[:], d_dy[ts(i, P), :])
            dy_sb.append(dyt)
        eh_sb = []
        for i in range(16):
            eht = wpool.tile([P, D], f32, tag=f"eh{i}", name=f"eh{i}")
            nc.sync.dma_start(eht[:], d_eh[ts(i, P), :])
            eh_sb.append(eht)
        cos_sb = []
        sin_sb = []
        for i in range(4):
            ct = wpool.tile([P, T], f32, tag=f"cos{i}", name=f"cos{i}")
            nc.sync.dma_start(ct[:], d_cos[ts(i, P), :])
            cos_sb.append(ct)
        for i in range(4):
            st = wpool.tile([P, T], f32, tag=f"sin{i}", name=f"sin{i}")
            nc.sync.dma_start(st[:], d_sin[ts(i, P), :])
            sin_sb.append(st)
        ew_sb = []
        ro_sb = []
        for i in range(ND):
            ewt = wpool.tile([P, D], f32, tag=f"ew{i}", name=f"ew{i}")
            nc.sync.dma_start(ewt[:], d_ew[ts(i, P), :])
            ew_sb.append(ewt)
        for i in range(ND):
            rot = wpool.tile([P, V], f32, tag=f"ro{i}", name=f"ro{i}")
            nc.sync.dma_start(rot[:], d_ro[ts(i, P), :])
            ro_sb.append(rot)
        ident = wpool.tile([P, P], f32, tag="ident", name="ident")
        make_identity(nc, ident)
        epsc = wpool.tile([P, 1], f32, tag="epsc", name="epsc")
        nc.gpsimd.memset(epsc[:], EPS)
        zeros512 = wpool.tile([P, 512], f32, tag="zeros512", name="zeros512")
        nc.gpsimd.memset(zeros512[:], 0.0)

        # ---- persistent activations ----
        v_sb = [vpool.tile([P, D], f32, tag=f"v{m}", name=f"v{m}") for m in range(NT)]
        vT_sb = [vpool.tile([P, T], f32, tag=f"vT{k}", name=f"vT{k}") for k in range(ND)]

        def layer_norm(src_ap, dst_ap):
            st6 = stpool.tile([P, 6], f32, tag="st6", name="st6")
            nc.vector.bn_stats(st6[:], src_ap)
            mv = stpool.tile([P, 2], f32, tag="mv", name="mv")
            nc.vector.bn_aggr(mv[:], st6[:])
            sd = stpool.tile([P, 1], f32, tag="sd", name="sd")
            nc.scalar.activation(sd[:], mv[:, 1:2], AF.Sqrt, bias=epsc[:], scale=1.0)
            rstd = stpool.tile([P, 1], f32, tag="rstd", name="rstd")
            nc.vector.reciprocal(rstd[:], sd[:])
            nmr = stpool.tile([P, 1], f32, tag="nmr", name="nmr")
            nc.vector.scalar_tensor_tensor(
                nmr[:], mv[:, 0:1], -1.0, rstd[:], op0=ALU.mult, op1=ALU.mult)
            nc.scalar.activation(dst_ap, src_ap, AF.Identity,
                                 bias=nmr[:], scale=rstd[:])

        def transpose_v():
            # vT[d, t] <- v[t, d]
            for m in range(NT):
                for d in range(ND):
                    tps = psA.tile([P, P], f32, tag="psA", name="tps")
                    nc.tensor.transpose(tps[:], v_sb[m][:, ts(d, P)], ident[:])
                    nc.scalar.copy(vT_sb[d][:, ts(m, P)], tps[:])

        # ---- embedding: v0 = LN(onehot @ emb_w) ----
        oh_sb = []
        for k in range(ND):
            oht = spool.tile([P, T], f32, tag="score", name=f"oh{k}")
            nc.sync.dma_start(oht[:], d_oh[ts(k, P), :])
            oh_sb.append(oht)
        for m in range(NT):
            eps_t = psA.tile([P, D], f32, tag="psA", name="embps")
            for k in range(ND):
                nc.tensor.matmul(eps_t[:], oh_sb[k][:, ts(m, P)], ew_sb[k][:],
                                 start=(k == 0), stop=(k == ND - 1))
            emb_t = lnpool.tile([P, D], f32, tag="w", name="embt")
            nc.scalar.copy(emb_t[:], eps_t[:])
            layer_norm(emb_t[:], v_sb[m][:])
        transpose_v()

        rg = [[0, 1], [2, 3], [4, 5], [6, 7]]

        for layer in range(L):
            z_sb = [zpool.tile([P, T], f32, tag=f"z{i}", name=f"z{i}_{layer}")
                    for i in range(2)]
            for j in range(2):  # local head index
                # ---- A: xT = relu(Dx^T @ vT), interleaved with RoPE ----
                # xT tiles are spilled to DRAM after RoPE and reloaded in the
                # D phase (SBUF is the binding constraint).
                xd = [dpool.tile([P, T], f32, tag="xspill", bufs=8,
                                 name=f"xd{mm}") for mm in range(NDh)]
                xr = [None] * NDh
                for m in range(4):
                    pair = []
                    for mm in (m, m + 4):
                        xps = psA.tile([P, T], f32, tag="psA", name="xps")
                        for n in range(2):
                            for k in range(ND):
                                nc.tensor.matmul(
                                    xps[:, ts(n, 512)],
                                    dx_sb[2 * j + k][:, ts(mm, P)],
                                    vT_sb[k][:, ts(n, 512)],
                                    start=(k == 0), stop=(k == ND - 1))
                        xt = xpool.tile([P, T], f32, tag="xT", name=f"xT{mm}")
                        nc.scalar.activation(xt[:], xps[:], AF.Relu)
                        pair.append(xt)
                    # RoPE on the (m, m+4) pair
                    cm, sm = cos_sb[m], sin_sb[m]
                    lo, hi = pair
                    xrl = xrpool.tile([P, T], f32, tag="xr", name=f"xr{m}")
                    xrh = xrpool.tile([P, T], f32, tag="xr", name=f"xr{m + 4}")
                    t1 = rtpool.tile([P, T], f32, tag="ropetmp", name="rt1")
                    nc.vector.tensor_mul(t1[:], hi[:], sm[:])
                    nc.vector.tensor_mul(xrl[:], lo[:], cm[:])
                    nc.vector.tensor_sub(xrl[:], xrl[:], t1[:])
                    t2 = rtpool.tile([P, T], f32, tag="ropetmp", name="rt2")
                    nc.vector.tensor_mul(t2[:], lo[:], sm[:])
                    nc.vector.tensor_mul(xrh[:], hi[:], cm[:])
                    nc.vector.tensor_add(xrh[:], xrh[:], t2[:])
                    xr[m], xr[m + 4] = xrl, xrh
                    nc.sync.dma_start(xd[m][:], lo[:])
                    nc.sync.dma_start(xd[m + 4][:], hi[:])

                # ---- C: S = xr @ xr^T streamed; aT += v^T @ S ----
                aT_ps = [psB.tile([P, T], f32, tag="psB", name=f"aTps{m}")
                         for m in range(ND)]
                for k in range(NT):
                    sps = psA.tile([P, T], f32, tag="psA", name="sps")
                    for n in range(2):
                        for kk in range(NDh):
                            nc.tensor.matmul(
                                sps[:, ts(n, 512)],
                                xr[kk][:, ts(k, P)],
                                xr[kk][:, ts(n, 512)],
                                start=(kk == 0), stop=(kk == NDh - 1))
                    s_sb = spool.tile([P, T], f32, tag="score", name=f"s{k}")
                    nc.scalar.copy(s_sb[:], sps[:])
                    for m in range(ND):
                        for n in range(2):
                            nc.tensor.matmul(
                                aT_ps[m][:, ts(n, 512)],
                                v_sb[k][:, ts(m, P)],
                                s_sb[:, ts(n, 512)],
                                start=(k == 0), stop=(k == NT - 1))
                aT = []
                for m in range(ND):
                    at = apool.tile([P, T], f32, tag="aT", name=f"aT{m}")
                    nc.scalar.copy(at[:], aT_ps[m][:])
                    aT.append(at)

                # ---- D/E: yT = relu(Dy^T @ aT) * xT ; z += yT^T @ E_h ----
                # z [T,D] tiles pack 4 token-blocks of 256 per psum tile (two
                # per bank), so open each bank once with a zeroing matmul and
                # accumulate everything with start=False (start=True clears
                # has_written for the whole bank).
                z_ps = [psB.tile([P, T], f32, tag="psB", name=f"zps{i}")
                        for i in range(2)]
                for i in range(2):
                    for half in range(2):
                        nc.tensor.matmul(
                            z_ps[i][:, ts(half, 512)], ident[:], zeros512[:],
                            start=True, stop=False)
                for k in range(NDh):
                    yps = psA.tile([P, T], f32, tag="psA", name="yps")
                    for n in range(2):
                        for kk in range(ND):
                            nc.tensor.matmul(
                                yps[:, ts(n, 512)],
                                dy_sb[2 * j + kk][:, ts(k, P)],
                                aT[kk][:, ts(n, 512)],
                                start=(kk == 0), stop=(kk == ND - 1))
                    xt2 = xpool.tile([P, T], f32, tag="xT", name=f"xre{k}")
                    nc.sync.dma_start(xt2[:], xd[k][:])
                    y_sb = ypool.tile([P, T], f32, tag="yT", name=f"y{k}")
                    # y = max(yps, 0) * x   (fused relu+mul on DVE)
                    nc.vector.scalar_tensor_tensor(
                        y_sb[:], yps[:], 0.0, xt2[:], op0=ALU.max, op1=ALU.mult)
                    for m in range(NT):
                        nc.tensor.matmul(
                            z_ps[m // 4][:, ts(m % 4, D)],
                            y_sb[:, ts(m, P)],
                            eh_sb[8 * j + k][:],
                            start=False,
                            stop=(k == NDh - 1 and m % 2 == 1))
                if j == 0:
                    for i in range(2):
                        nc.scalar.copy(z_sb[i][:], z_ps[i][:])
                else:
                    for i in range(2):
                        nc.vector.scalar_tensor_tensor(
                            z_sb[i][:], z_ps[i][:], 0.0, z_sb[i][:],
                            op0=ALU.add, op1=ALU.add)

            # ---- boundary: AllReduce(z) over the core pair, then v update ----
            zin = dpool.tile([2 * P, T], f32, tag="zin", name=f"zin{layer}")
            zout = dpool.tile([2 * P, T], f32, tag="zout", name=f"zout{layer}")
            for i in range(2):
                nc.sync.dma_start(zin[ts(i, P), :], z_sb[i][:])
            nc.gpsimd.collective_compute(
                "AllReduce", mybir.AluOpType.add,
                ins=[zin.opt()], outs=[zout.opt()], replica_groups=rg)
            # reuse the z slots for the reduced result (z is dead after the
            # DMA into zin)
            zr_sb = [zpool.tile([P, T], f32, tag=f"z{i}", name=f"zr{i}_{layer}")
                     for i in range(2)]
            for i in range(2):
                nc.sync.dma_start(zr_sb[i][:], zout[ts(i, P), :])
            for m in range(NT):
                zb = zr_sb[m // 4][:, ts(m % 4, D)]
                u = lnpool.tile([P, D], f32, tag="u", name=f"u{m}")
                layer_norm(zb, u[:])
                w = lnpool.tile([P, D], f32, tag="w", name=f"w{m}")
                nc.vector.tensor_add(w[:], v_sb[m][:], u[:])
                layer_norm(w[:], v_sb[m][:])
            transpose_v()

        # ---- readout ----
        for m in range(NT):
            rps = psA.tile([P, V], f32, tag="psA", name="rps")
            for k in range(ND):
                nc.tensor.matmul(rps[:], vT_sb[k][:, ts(m, P)], ro_sb[k][:],
                                 start=(k == 0), stop=(k == ND - 1))
            o_sb = lnpool.tile([P, V], f32, tag="o", name=f"o{m}")
            nc.scalar.copy(o_sb[:], rps[:])
            nc.sync.dma_start(d_out[ts(m, P), :], o_sb[:])

    nc.compile()
    return nc


def _get_program():
    if "nc" not in _CACHE:
        _CACHE["nc"] = _build_program()
    return _CACHE["nc"]


def _rope_tables():
    inv = (1.0 / (10000.0 ** (np.arange(0, Dh, 2, dtype=np.float32) / Dh)))
    tt = np.arange(T, dtype=np.float32)
    freqs = np.outer(tt, inv).astype(np.float32)  # [T, Dh/2]
    cosT = np.ascontiguousarray(np.cos(freqs).T, dtype=np.float32)
    sinT = np.ascontiguousarray(np.sin(freqs).T, dtype=np.float32)
    return cosT, sinT


def kernel(**inputs):
    global LAST_RESULT
    from concourse import bass_utils

    tokens = np.asarray(inputs["tokens"])
    emb_w = np.ascontiguousarray(inputs["emb_w"], dtype=np.float32)
    E = np.ascontiguousarray(inputs["E"], dtype=np.float32)
    Dx = np.ascontiguousarray(inputs["Dx"], dtype=np.float32)
    Dy = np.ascontiguousarray(inputs["Dy"], dtype=np.float32)
    readout = np.ascontiguousarray(inputs["readout"], dtype=np.float32)

    cosT, sinT = _rope_tables()

    in_maps = []
    for c in range(NCORES):
        b, hp = c // 2, c % 2
        oh = np.zeros((V, T), dtype=np.float32)
        oh[np.asarray(tokens[b], dtype=np.int64), np.arange(T)] = 1.0
        in_maps.append({
            "onehotT": oh,
            "emb_w": emb_w,
            "dx": np.ascontiguousarray(
                Dx[2 * hp:2 * hp + 2].reshape(2 * D, Dh)),
            "dy": np.ascontiguousarray(
                Dy[2 * hp:2 * hp + 2].reshape(2 * D, Dh)),
            "eh": np.ascontiguousarray(E[2 * hp * Dh:(2 * hp + 2) * Dh]),
            "cosT": cosT,
            "sinT": sinT,
            "readout": readout,
        })

    nc = _get_program()
    res = bass_utils.run_bass_kernel_spmd(
        nc, in_maps, core_ids=list(range(NCORES)),
        trace=bool(int(os.environ.get("KERNEL_TRACE", "0"))))
    LAST_RESULT = res
    out = np.stack([res.results[2 * b]["out"] for b in range(B)], axis=0)
    return out


# revision 14
# speedup vs baseline: 2.3672x; 2.3672x over previous
# Trainium2 Bass kernel for nn_BDH_66056597013022 (dense_transformer).
#
# Model (per reference):
#   v = LN(emb_w[tokens])                                  [B,T,D]
#   6x: x  = relu(v @ Dx_h)            per head            [B,H,T,Dh]
#       xr = RoPE(x)
#       S  = xr @ xr^T                 (no softmax)        [B,H,T,T]
#       a  = S @ v                                         [B,H,T,D]
#       y  = relu(a @ Dy_h) * x                            [B,H,T,Dh]
#       v  = LN(v + LN(concat_h(y) @ E))
#   out = v @ readout                                      [B,T,V]
#
# Shapes: B=4 T=1024 H=4 N=4096 D=256 L=6 V=256, Dh=N/H=1024.
#
# Sharding (8 cores): core c -> batch b=c//2, head-pair hp=c%2 (heads 2hp,2hp+1).
# All per-head work is local; the only cross-core coupling is the head-sum in
# z = y @ E, handled with a 2-rank AllReduce per layer between cores {2b,2b+1}.
# Both cores of a pair then redundantly compute the LN/v-update, so the whole
# forward stays on-device; even cores' outputs are returned.
#
# On-chip layouts per core (SBUF):
#   v    [T,D]   8 tiles [128,256]   (token rows on partitions)
#   vT   [D,T]   2 tiles [128,1024]  (for contractions over D)
#   xT,xrT,yT [Dh,T] 8 tiles [128,1024] each head (Dh on partitions)
#   S streamed per 128-row block [128,1024]; aT [D,T] 2 tiles.
# All matmuls are out = lhsT.T @ rhs with K<=128 on partitions; S is
# numerically symmetric so its [t,s] tiles serve as [s,t] operands directly.
#
# Matmul operands are float32r (TF32-like: ~1e-4 rounding, 1 cycle/row at
# N>=256 vs 4 for fp32). f32r operands must be produced by a compute
# instruction that rounds (ACT/DVE write with f32r out dtype); DMA-produced
# weights get a one-time ACT round-copy. PSUM accumulation stays fp32.

import os
import numpy as np

B, T, H, N, D, L, V = 4, 1024, 4, 4096, 256, 6, 256
Dh = N // H
EPS = 1e-5
NCORES = 8
P = 128
NT = T // P  # 8 tiles of tokens
ND = D // P  # 2 tiles of model dim
NDh = Dh // P  # 8 tiles of head dim

_CACHE = {}
LAST_RESULT = None


def _build_program():
    from contextlib import ExitStack

    import concourse.bass as bass
    import concourse.bacc as bacc
    import concourse.tile as tile
    import concourse.mybir as mybir
    from concourse.masks import make_identity

    f32 = mybir.dt.float32
    f32r = mybir.dt.float32r
    AF = mybir.ActivationFunctionType
    ALU = mybir.AluOpType
    ts = bass.ts

    nc = bacc.Bacc("TRN2", target_bir_lowering=False, debug=False,
                   enable_asserts=False, num_devices=NCORES)

    d_oh = nc.dram_tensor("onehotT", [V, T], f32, kind="ExternalInput").ap()
    d_ew = nc.dram_tensor("emb_w", [V, D], f32, kind="ExternalInput").ap()
    d_dx = nc.dram_tensor("dx", [2 * D, Dh], f32, kind="ExternalInput").ap()
    d_dy = nc.dram_tensor("dy", [2 * D, Dh], f32, kind="ExternalInput").ap()
    d_eh = nc.dram_tensor("eh", [2 * Dh, D], f32, kind="ExternalInput").ap()
    d_cos = nc.dram_tensor("cosT", [Dh // 2, T], f32, kind="ExternalInput").ap()
    d_sin = nc.dram_tensor("sinT", [Dh // 2, T], f32, kind="ExternalInput").ap()
    d_ro = nc.dram_tensor("readout", [D, V], f32, kind="ExternalInput").ap()
    d_out = nc.dram_tensor("out", [T, V], f32, kind="ExternalOutput").ap()

    with tile.TileContext(nc) as tc, ExitStack() as ctx:
        wpool = ctx.enter_context(tc.tile_pool(name="weights", bufs=1))
        vpool = ctx.enter_context(tc.tile_pool(name="vpool", bufs=1))
        xpool = ctx.enter_context(tc.tile_pool(name="xpool", bufs=4))
        xrpool = ctx.enter_context(tc.tile_pool(name="xrpool", bufs=8))
        spool = ctx.enter_context(tc.tile_pool(name="spool", bufs=2))
        apool = ctx.enter_context(tc.tile_pool(name="apool", bufs=2))
        ypool = ctx.enter_context(tc.tile_pool(name="ypool", bufs=3))
        zpool = ctx.enter_context(tc.tile_pool(name="zpool", bufs=1))
        lnpool = ctx.enter_context(tc.tile_pool(name="lnpool", bufs=3))
        stpool = ctx.enter_context(tc.tile_pool(name="stpool", bufs=4))
        rtpool = ctx.enter_context(tc.tile_pool(name="rtpool", bufs=2))
        psA = ctx.enter_context(tc.tile_pool(name="psA", bufs=2, space="PSUM"))
        psB = ctx.enter_context(tc.tile_pool(name="psB", bufs=2, space="PSUM"))
        dpool = ctx.enter_context(tc.tile_pool(name="drampool", bufs=2, space="DRAM"))

        # ---- persistent weights (DMA to staging, then round-copy to f32r) ----
        def load_rounded(dram_ap, n_tiles, width, tag):
            tiles = []
            for i in range(n_tiles):
                stg = rtpool.tile([P, T], f32, tag="ropetmp", name=f"stg_{tag}{i}")
                nc.sync.dma_start(stg[:, :width], dram_ap[ts(i, P), :])
                wt = wpool.tile([P, width], f32r, tag=f"{tag}{i}", name=f"{tag}{i}")
                nc.scalar.copy(wt[:], stg[:, :width])
                tiles.append(wt)
            return tiles

        dx_sb = load_rounded(d_dx, 4, Dh, "dx")
        dy_sb = load_rounded(d_dy, 4, Dh, "dy")
        eh_sb = load_rounded(d_eh, 16, D, "eh")
        ew_sb = load_rounded(d_ew, ND, D, "ew")
        ro_sb = load_rounded(d_ro, ND, V, "ro")
        cos_sb = []
        sin_sb = []
        for i in range(4):
            ct = wpool.tile([P, T], f32, tag=f"cos{i}", name=f"cos{i}")
            nc.sync.dma_start(ct[:], d_cos[ts(i, P), :])
            cos_sb.append(ct)
        for i in range(4):
            st = wpool.tile([P, T], f32, tag=f"sin{i}", name=f"sin{i}")
            nc.sync.dma_start(st[:], d_sin[ts(i, P), :])
            sin_sb.append(st)
        ident = wpool.tile([P, P], f32, tag="ident", name="ident")
        make_identity(nc, ident)
        epsc = wpool.tile([P, 1], f32, tag="epsc", name="epsc")
        nc.gpsimd.memset(epsc[:], EPS)
        # f32r zeros for opening z psum banks (rounding producer = ACT)
        zeros512 = wpool.tile([P, 512], f32r, tag="zeros512", name="zeros512")
        nc.scalar.mul(zeros512[:], cos_sb[0][:, 0:512], 0.0)

        # ---- persistent activations ----
        v_sb = [vpool.tile([P, D], f32r, tag=f"v{m}", name=f"v{m}")
                for m in range(NT)]
        vT_sb = [vpool.tile([P, T], f32r, tag=f"vT{k}", name=f"vT{k}")
                 for k in range(ND)]

        def layer_norm(src_ap, dst_ap):
            st6 = stpool.tile([P, 6], f32, tag="st6", name="st6")
            nc.vector.bn_stats(st6[:], src_ap)
            mv = stpool.tile([P, 2], f32, tag="mv", name="mv")
            nc.vector.bn_aggr(mv[:], st6[:])
            sd = stpool.tile([P, 1], f32, tag="sd", name="sd")
            nc.scalar.activation(sd[:], mv[:, 1:2], AF.Sqrt, bias=epsc[:], scale=1.0)
            rstd = stpool.tile([P, 1], f32, tag="rstd", name="rstd")
            nc.vector.reciprocal(rstd[:], sd[:])
            nmr = stpool.tile([P, 1], f32, tag="nmr", name="nmr")
            nc.vector.scalar_tensor_tensor(
                nmr[:], mv[:, 0:1], -1.0, rstd[:], op0=ALU.mult, op1=ALU.mult)
            nc.scalar.activation(dst_ap, src_ap, AF.Identity,
                                 bias=nmr[:], scale=rstd[:])

        def transpose_v():
            # vT[d, t] <- v[t, d]; transpose runs as plain fp32 (bitcast view),
            # the ACT eviction rounds into the f32r vT tile.
            for m in range(NT):
                for d in range(ND):
                    tps = psA.tile([P, P], f32, tag="psA", name="tps")
                    nc.tensor.transpose(
                        tps[:], v_sb[m][:, ts(d, P)].bitcast(f32), ident[:])
                    nc.scalar.copy(vT_sb[d][:, ts(m, P)], tps[:])

        # ---- embedding: v0 = LN(onehot @ emb_w) ----
        oh_sb = []
        for k in range(ND):
            stg = rtpool.tile([P, T], f32, tag="ropetmp", name=f"stg_oh{k}")
            nc.sync.dma_start(stg[:], d_oh[ts(k, P), :])
            oht = spool.tile([P, T], f32r, tag="score", name=f"oh{k}")
            nc.scalar.copy(oht[:], stg[:])
            oh_sb.append(oht)
        for m in range(NT):
            eps_t = psA.tile([P, D], f32, tag="psA", name="embps")
            for k in range(ND):
                nc.tensor.matmul(eps_t[:], oh_sb[k][:, ts(m, P)], ew_sb[k][:],
                                 start=(k == 0), stop=(k == ND - 1))
            emb_t = lnpool.tile([P, D], f32, tag="w", name="embt")
            nc.scalar.copy(emb_t[:], eps_t[:])
            layer_norm(emb_t[:], v_sb[m][:])
        transpose_v()

        rg = [[0, 1], [2, 3], [4, 5], [6, 7]]

        for layer in range(L):
            z_sb = [zpool.tile([P, T], f32, tag=f"z{i}", name=f"z{i}_{layer}")
                    for i in range(2)]
            for j in range(2):  # local head index
                # ---- A: xT = relu(Dx^T @ vT), interleaved with RoPE ----
                # xT tiles are spilled to DRAM after RoPE and reloaded in the
                # D phase (SBUF is the binding constraint).
                xd = [dpool.tile([P, T], f32, tag="xspill", bufs=8,
                                 name=f"xd{mm}") for mm in range(NDh)]
                xr = [None] * NDh
                for m in range(4):
                    pair = []
                    for mm in (m, m + 4):
                        xps = psA.tile([P, T], f32, tag="psA", name="xps")
                        for n in range(2):
                            for k in range(ND):
                                nc.tensor.matmul(
                                    xps[:, ts(n, 512)],
                                    dx_sb[2 * j + k][:, ts(mm, P)],
                                    vT_sb[k][:, ts(n, 512)],
                                    start=(k == 0), stop=(k == ND - 1))
                        xt = xpool.tile([P, T], f32, tag="xT", name=f"xT{mm}")
                        nc.scalar.activation(xt[:], xps[:], AF.Relu)
                        pair.append(xt)
                    # RoPE on the (m, m+4) pair; final DVE op rounds into the
                    # f32r xr tile (read back via fp32 bitcast for in-place op)
                    cm, sm = cos_sb[m], sin_sb[m]
                    lo, hi = pair
                    xrl = xrpool.tile([P, T], f32r, tag="xr", name=f"xr{m}")
                    xrh = xrpool.tile([P, T], f32r, tag="xr", name=f"xr{m + 4}")
                    t1 = rtpool.tile([P, T], f32, tag="ropetmp", name="rt1")
                    nc.vector.tensor_mul(t1[:], hi[:], sm[:])
                    nc.vector.tensor_mul(xrl[:], lo[:], cm[:])
                    nc.vector.tensor_sub(xrl[:], xrl[:].bitcast(f32), t1[:])
                    t2 = rtpool.tile([P, T], f32, tag="ropetmp", name="rt2")
                    nc.vector.tensor_mul(t2[:], lo[:], sm[:])
                    nc.vector.tensor_mul(xrh[:], hi[:], cm[:])
                    nc.vector.tensor_add(xrh[:], xrh[:].bitcast(f32), t2[:])
                    xr[m], xr[m + 4] = xrl, xrh
                    nc.sync.dma_start(xd[m][:], lo[:])
                    nc.sync.dma_start(xd[m + 4][:], hi[:])

                # ---- C: S = xr @ xr^T streamed; aT += v^T @ S ----
                aT_ps = [psB.tile([P, T], f32, tag="psB", name=f"aTps{m}")
                         for m in range(ND)]
                for k in range(NT):
                    sps = psA.tile([P, T], f32, tag="psA", name="sps")
                    for n in range(2):
                        for kk in range(NDh):
                            nc.tensor.matmul(
                                sps[:, ts(n, 512)],
                                xr[kk][:, ts(k, P)],
                                xr[kk][:, ts(n, 512)],
                                start=(kk == 0), stop=(kk == NDh - 1))
                    s_sb = spool.tile([P, T], f32r, tag="score", name=f"s{k}")
                    nc.scalar.copy(s_sb[:], sps[:])
                    for m in range(ND):
                        for n in range(2):
                            nc.tensor.matmul(
                                aT_ps[m][:, ts(n, 512)],
                                v_sb[k][:, ts(m, P)],
                                s_sb[:, ts(n, 512)],
                                start=(k == 0), stop=(k == NT - 1))
                aT = []
                for m in range(ND):
                    at = apool.tile([P, T], f32r, tag="aT", name=f"aT{m}")
                    nc.scalar.copy(at[:], aT_ps[m][:])
                    aT.append(at)

                # ---- D/E: yT = relu(Dy^T @ aT) * xT ; z += yT^T @ E_h ----
                # z [T,D] tiles pack 4 token-blocks of 256 per psum tile (two
                # per bank), so open each bank once with a zeroing matmul and
                # accumulate everything with start=False (start=True clears
                # has_written for the whole bank).
                z_ps = [psB.tile([P, T], f32, tag="psB", name=f"zps{i}")
                        for i in range(2)]
                for i in range(2):
                    for half in range(2):
                        nc.tensor.matmul(
                            z_ps[i][:, ts(half, 512)],
                            dx_sb[0][:, 0:P], zeros512[:],
                            start=True, stop=False)
                for k in range(NDh):
                    yps = psA.tile([P, T], f32, tag="psA", name="yps")
                    for n in range(2):
                        for kk in range(ND):
                            nc.tensor.matmul(
                                yps[:, ts(n, 512)],
                                dy_sb[2 * j + kk][:, ts(k, P)],
                                aT[kk][:, ts(n, 512)],
                                start=(kk == 0), stop=(kk == ND - 1))
                    xt2 = xpool.tile([P, T], f32, tag="xT", name=f"xre{k}")
                    nc.sync.dma_start(xt2[:], xd[k][:])
                    y_sb = ypool.tile([P, T], f32r, tag="yT", name=f"y{k}")
                    # y = max(yps, 0) * x   (fused relu+mul on DVE, f32r out)
                    nc.vector.scalar_tensor_tensor(
                        y_sb[:], yps[:], 0.0, xt2[:], op0=ALU.max, op1=ALU.mult)
                    for m in range(NT):
                        nc.tensor.matmul(
                            z_ps[m // 4][:, ts(m % 4, D)],
                            y_sb[:, ts(m, P)],
                            eh_sb[8 * j + k][:],
                            start=False,
                            stop=(k == NDh - 1 and m % 2 == 1))
                if j == 0:
                    for i in range(2):
                        nc.scalar.copy(z_sb[i][:], z_ps[i][:])
                else:
                    for i in range(2):
                        nc.vector.scalar_tensor_tensor(
                            z_sb[i][:], z_ps[i][:], 0.0, z_sb[i][:],
                            op0=ALU.add, op1=ALU.add)

            # ---- boundary: AllReduce(z) over the core pair, then v update ----
            zin = dpool.tile([2 * P, T], f32, tag="zin", name=f"zin{layer}")
            zout = dpool.tile([2 * P, T], f32, tag="zout", name=f"zout{layer}")
            for i in range(2):
                nc.sync.dma_start(zin[ts(i, P), :], z_sb[i][:])
            nc.gpsimd.collective_compute(
                "AllReduce", mybir.AluOpType.add,
                ins=[zin.opt()], outs=[zout.opt()], replica_groups=rg)
            # reuse the z slots for the reduced result (z is dead after the
            # DMA into zin)
            zr_sb = [zpool.tile([P, T], f32, tag=f"z{i}", name=f"zr{i}_{layer}")
                     for i in range(2)]
            for i in range(2):
                nc.sync.dma_start(zr_sb[i][:], zout[ts(i, P), :])
            for m in range(NT):
                zb = zr_sb[m // 4][:, ts(m % 4, D)]
                u = lnpool.tile([P, D], f32, tag="u", name=f"u{m}")
                layer_norm(zb, u[:])
                w = lnpool.tile([P, D], f32, tag="w", name=f"w{m}")
                nc.vector.tensor_add(w[:], v_sb[m][:].bitcast(f32), u[:])
                layer_norm(w[:], v_sb[m][:])
            transpose_v()

        # ---- readout ----
        for m in range(NT):
            rps = psA.tile([P, V], f32, tag="psA", name="rps")
            for k in range(ND):
                nc.tensor.matmul(rps[:], vT_sb[k][:, ts(m, P)], ro_sb[k][:],
                                 start=(k == 0), stop=(k == ND - 1))
            o_sb = lnpool.tile([P, V], f32, tag="o", name=f"o{m}")
            nc.scalar.copy(o_sb[:], rps[:])
            nc.sync.dma_start(d_out[ts(m, P), :], o_sb[:])

    nc.compile()
    return nc


def _get_program():
    if "nc" not in _CACHE:
        _CACHE["nc"] = _build_program()
    return _CACHE["nc"]


def _rope_tables():
    inv = (1.0 / (10000.0 ** (np.arange(0, Dh, 2, dtype=np.float32) / Dh)))
    tt = np.arange(T, dtype=np.float32)
    freqs = np.outer(tt, inv).astype(np.float32)  # [T, Dh/2]
    cosT = np.ascontiguousarray(np.cos(freqs).T, dtype=np.float32)
    sinT = np.ascontiguousarray(np.sin(freqs).T, dtype=np.float32)
    return cosT, sinT


def kernel(**inputs):
    global LAST_RESULT
    from concourse import bass_utils

    tokens = np.asarray(inputs["tokens"])
    emb_w = np.ascontiguousarray(inputs["emb_w"], dtype=np.float32)
    E = np.ascontiguousarray(inputs["E"], dtype=np.float32)
    Dx = np.ascontiguousarray(inputs["Dx"], dtype=np.float32)
    Dy = np.ascontiguousarray(inputs["Dy"], dtype=np.float32)
    readout = np.ascontiguousarray(inputs["readout"], dtype=np.float32)

    cosT, sinT = _rope_tables()

    in_maps = []
    for c in range(NCORES):
        b, hp = c // 2, c % 2
        oh = np.zeros((V, T), dtype=np.float32)
        oh[np.asarray(tokens[b], dtype=np.int64), np.arange(T)] = 1.0
        in_maps.append({
            "onehotT": oh,
            "emb_w": emb_w,
            "dx": np.ascontiguousarray(
                Dx[2 * hp:2 * hp + 2].reshape(2 * D, Dh)),
            "dy": np.ascontiguousarray(
                Dy[2 * hp:2 * hp + 2].reshape(2 * D, Dh)),
            "eh": np.ascontiguousarray(E[2 * hp * Dh:(2 * hp + 2) * Dh]),
            "cosT": cosT,
            "sinT": sinT,
            "readout": readout,
        })

    nc = _get_program()
    res = bass_utils.run_bass_kernel_spmd(
        nc, in_maps, core_ids=list(range(NCORES)),
        trace=bool(int(os.environ.get("KERNEL_TRACE", "0"))))
    LAST_RESULT = res
    out = np.stack([res.results[2 * b]["out"] for b in range(B)], axis=0)
    return out


# revision 18
# speedup vs baseline: 2.4310x; 1.0269x over previous
# Trainium2 Bass kernel for nn_BDH_66056597013022 (dense_transformer).
#
# Model (per reference):
#   v = LN(emb_w[tokens])                                  [B,T,D]
#   6x: x  = relu(v @ Dx_h)            per head            [B,H,T,Dh]
#       xr = RoPE(x)
#       S  = xr @ xr^T                 (no softmax)        [B,H,T,T]
#       a  = S @ v                                         [B,H,T,D]
#       y  = relu(a @ Dy_h) * x                            [B,H,T,Dh]
#       v  = LN(v + LN(concat_h(y) @ E))
#   out = v @ readout                                      [B,T,V]
#
# Shapes: B=4 T=1024 H=4 N=4096 D=256 L=6 V=256, Dh=N/H=1024.
#
# Sharding (8 cores): core c -> batch b=c//2, head-pair hp=c%2 (heads 2hp,2hp+1).
# All per-head work is local; the only cross-core coupling is the head-sum in
# z = y @ E, handled with a 2-rank AllReduce per layer between cores {2b,2b+1}.
# Both cores of a pair then redundantly compute the LN/v-update, so the whole
# forward stays on-device; even cores' outputs are returned.
#
# On-chip layouts per core (SBUF):
#   v    [T,D]   8 tiles [128,256]   (token rows on partitions)
#   vT   [D,T]   2 tiles [128,1024]  (for contractions over D)
#   xT,xrT,yT [Dh,T] 8 tiles [128,1024] each head (Dh on partitions)
#   S streamed per 128-row block [128,1024]; aT [D,T] 2 tiles.
# All matmuls are out = lhsT.T @ rhs with K<=128 on partitions; S is
# numerically symmetric so its [t,s] tiles serve as [s,t] operands directly.
#
# Matmul operands are float32r (TF32-like: ~1e-4 rounding, 1 cycle/row at
# N>=256 vs 4 for fp32). f32r operands must be produced by a compute
# instruction that rounds (ACT/DVE write with f32r out dtype); DMA-produced
# weights get a one-time ACT round-copy. PSUM accumulation stays fp32.

import os
import numpy as np

B, T, H, N, D, L, V = 4, 1024, 4, 4096, 256, 6, 256
Dh = N // H
EPS = 1e-5
NCORES = 8
P = 128
NT = T // P  # 8 tiles of tokens
ND = D // P  # 2 tiles of model dim
NDh = Dh // P  # 8 tiles of head dim

_CACHE = {}
LAST_RESULT = None


def _build_program():
    from contextlib import ExitStack

    import concourse.bass as bass
    import concourse.bacc as bacc
    import concourse.tile as tile
    import concourse.mybir as mybir
    from concourse.masks import make_identity

    f32 = mybir.dt.float32
    f32r = mybir.dt.float32r
    AF = mybir.ActivationFunctionType
    ALU = mybir.AluOpType
    ts = bass.ts

    nc = bacc.Bacc("TRN2", target_bir_lowering=False, debug=False,
                   enable_asserts=False, num_devices=NCORES)

    d_oh = nc.dram_tensor("onehotT", [V, T], f32, kind="ExternalInput").ap()
    d_ew = nc.dram_tensor("emb_w", [V, D], f32, kind="ExternalInput").ap()
    d_dx = nc.dram_tensor("dx", [2 * D, Dh], f32, kind="ExternalInput").ap()
    d_dy = nc.dram_tensor("dy", [2 * D, Dh], f32, kind="ExternalInput").ap()
    d_eh = nc.dram_tensor("eh", [2 * Dh, D], f32, kind="ExternalInput").ap()
    d_cos = nc.dram_tensor("cosT", [Dh // 2, T], f32, kind="ExternalInput").ap()
    d_sin = nc.dram_tensor("sinT", [Dh // 2, T], f32, kind="ExternalInput").ap()
    d_ro = nc.dram_tensor("readout", [D, V], f32, kind="ExternalInput").ap()
    d_out = nc.dram_tensor("out", [T, V], f32, kind="ExternalOutput").ap()

    with tile.TileContext(nc) as tc, ExitStack() as ctx:
        wpool = ctx.enter_context(tc.tile_pool(name="weights", bufs=1))
        vpool = ctx.enter_context(tc.tile_pool(name="vpool", bufs=1))
        xpool = ctx.enter_context(tc.tile_pool(name="xpool", bufs=4))
        xrpool = ctx.enter_context(tc.tile_pool(name="xrpool", bufs=8))
        spool = ctx.enter_context(tc.tile_pool(name="spool", bufs=2))
        apool = ctx.enter_context(tc.tile_pool(name="apool", bufs=2))
        ypool = ctx.enter_context(tc.tile_pool(name="ypool", bufs=3))
        zpool = ctx.enter_context(tc.tile_pool(name="zpool", bufs=1))
        lnpool = ctx.enter_context(tc.tile_pool(name="lnpool", bufs=3))
        stpool = ctx.enter_context(tc.tile_pool(name="stpool", bufs=4))
        rtpool = ctx.enter_context(tc.tile_pool(name="rtpool", bufs=2))
        psA = ctx.enter_context(tc.tile_pool(name="psA", bufs=2, space="PSUM"))
        psB = ctx.enter_context(tc.tile_pool(name="psB", bufs=2, space="PSUM"))
        dpool = ctx.enter_context(tc.tile_pool(name="drampool", bufs=2, space="DRAM"))

        # ---- persistent weights (DMA to staging, then round-copy to f32r) ----
        # Staging cycles through several big pools so the DMAs and round
        # copies pipeline instead of ping-ponging through two slots.
        _stage_slots = [(rtpool, "ropetmp"), (ypool, "yT"), (apool, "aT"),
                        (spool, "score"), (rtpool, "ropetmp"), (ypool, "yT"),
                        (apool, "aT"), (ypool, "yT")]
        _stage_i = [0]

        def load_rounded(dram_ap, n_tiles, width, tag):
            tiles = []
            for i in range(n_tiles):
                pool, ptag = _stage_slots[_stage_i[0] % len(_stage_slots)]
                _stage_i[0] += 1
                stg = pool.tile([P, T], f32, tag=ptag, name=f"stg_{tag}{i}")
                nc.sync.dma_start(stg[:, :width], dram_ap[ts(i, P), :])
                wt = wpool.tile([P, width], f32r, tag=f"{tag}{i}", name=f"{tag}{i}")
                nc.scalar.copy(wt[:], stg[:, :width])
                tiles.append(wt)
            return tiles

        dx_sb = load_rounded(d_dx, 4, Dh, "dx")
        dy_sb = load_rounded(d_dy, 4, Dh, "dy")
        eh_sb = load_rounded(d_eh, 16, D, "eh")
        ew_sb = load_rounded(d_ew, ND, D, "ew")
        ro_sb = load_rounded(d_ro, ND, V, "ro")
        cos_sb = []
        sin_sb = []
        for i in range(4):
            ct = wpool.tile([P, T], f32, tag=f"cos{i}", name=f"cos{i}")
            nc.sync.dma_start(ct[:], d_cos[ts(i, P), :])
            cos_sb.append(ct)
        for i in range(4):
            st = wpool.tile([P, T], f32, tag=f"sin{i}", name=f"sin{i}")
            nc.sync.dma_start(st[:], d_sin[ts(i, P), :])
            sin_sb.append(st)
        ident = wpool.tile([P, P], f32, tag="ident", name="ident")
        make_identity(nc, ident)
        epsc = wpool.tile([P, 1], f32, tag="epsc", name="epsc")
        nc.gpsimd.memset(epsc[:], EPS)

        # ---- persistent activations ----
        v_sb = [vpool.tile([P, D], f32r, tag=f"v{m}", name=f"v{m}")
                for m in range(NT)]
        vT_sb = [vpool.tile([P, T], f32r, tag=f"vT{k}", name=f"vT{k}")
                 for k in range(ND)]

        def layer_norm(src_ap, dst_ap):
            st6 = stpool.tile([P, 6], f32, tag="st6", name="st6")
            nc.vector.bn_stats(st6[:], src_ap)
            mv = stpool.tile([P, 2], f32, tag="mv", name="mv")
            nc.vector.bn_aggr(mv[:], st6[:])
            sd = stpool.tile([P, 1], f32, tag="sd", name="sd")
            nc.scalar.activation(sd[:], mv[:, 1:2], AF.Sqrt, bias=epsc[:], scale=1.0)
            rstd = stpool.tile([P, 1], f32, tag="rstd", name="rstd")
            nc.vector.reciprocal(rstd[:], sd[:])
            nmr = stpool.tile([P, 1], f32, tag="nmr", name="nmr")
            nc.vector.scalar_tensor_tensor(
                nmr[:], mv[:, 0:1], -1.0, rstd[:], op0=ALU.mult, op1=ALU.mult)
            nc.scalar.activation(dst_ap, src_ap, AF.Identity,
                                 bias=nmr[:], scale=rstd[:])

        def transpose_v():
            # vT[d, t] <- v[t, d]; transpose runs as plain fp32 (bitcast view),
            # the ACT eviction rounds into the f32r vT tile.
            for m in range(NT):
                for d in range(ND):
                    tps = psA.tile([P, P], f32, tag="psA", name="tps")
                    nc.tensor.transpose(
                        tps[:], v_sb[m][:, ts(d, P)].bitcast(f32), ident[:])
                    nc.scalar.copy(vT_sb[d][:, ts(m, P)], tps[:])

        # ---- embedding: v0 = LN(onehot @ emb_w) ----
        oh_sb = []
        for k in range(ND):
            stg = rtpool.tile([P, T], f32, tag="ropetmp", name=f"stg_oh{k}")
            nc.sync.dma_start(stg[:], d_oh[ts(k, P), :])
            oht = spool.tile([P, T], f32r, tag="score", name=f"oh{k}")
            nc.scalar.copy(oht[:], stg[:])
            oh_sb.append(oht)
        for m in range(NT):
            eps_t = psA.tile([P, D], f32, tag="psA", name="embps")
            for k in range(ND):
                nc.tensor.matmul(eps_t[:], oh_sb[k][:, ts(m, P)], ew_sb[k][:],
                                 start=(k == 0), stop=(k == ND - 1))
            emb_t = lnpool.tile([P, D], f32, tag="w", name="embt")
            nc.scalar.copy(emb_t[:], eps_t[:])
            layer_norm(emb_t[:], v_sb[m][:])
        transpose_v()

        rg = [[0, 1], [2, 3], [4, 5], [6, 7]]

        for layer in range(L):
            z_sb = [zpool.tile([P, T], f32, tag=f"z{i}", name=f"z{i}_{layer}")
                    for i in range(2)]
            for j in range(2):  # local head index
                # ---- A: xT = relu(Dx^T @ vT), interleaved with RoPE ----
                # xT tiles are spilled to DRAM after RoPE and reloaded in the
                # D phase (SBUF is the binding constraint).
                xd = [dpool.tile([P, T], f32, tag="xspill", bufs=8,
                                 name=f"xd{mm}") for mm in range(NDh)]
                xr = [None] * NDh
                for m in range(4):
                    pair = []
                    for mm in (m, m + 4):
                        xps = psA.tile([P, T], f32, tag="psA", name="xps")
                        for n in range(2):
                            for k in range(ND):
                                nc.tensor.matmul(
                                    xps[:, ts(n, 512)],
                                    dx_sb[2 * j + k][:, ts(mm, P)],
                                    vT_sb[k][:, ts(n, 512)],
                                    start=(k == 0), stop=(k == ND - 1))
                        xt = xpool.tile([P, T], f32, tag="xT", name=f"xT{mm}")
                        nc.scalar.activation(xt[:], xps[:], AF.Relu)
                        pair.append(xt)
                    # RoPE on the (m, m+4) pair; final DVE op rounds into the
                    # f32r xr tile (read back via fp32 bitcast for in-place op)
                    cm, sm = cos_sb[m], sin_sb[m]
                    lo, hi = pair
                    xrl = xrpool.tile([P, T], f32r, tag="xr", name=f"xr{m}")
                    xrh = xrpool.tile([P, T], f32r, tag="xr", name=f"xr{m + 4}")
                    t1 = rtpool.tile([P, T], f32, tag="ropetmp", name="rt1")
                    nc.vector.tensor_mul(t1[:], hi[:], sm[:])
                    nc.vector.tensor_mul(xrl[:], lo[:], cm[:])
                    nc.vector.tensor_sub(xrl[:], xrl[:].bitcast(f32), t1[:])
                    t2 = rtpool.tile([P, T], f32, tag="ropetmp", name="rt2")
                    nc.vector.tensor_mul(t2[:], lo[:], sm[:])
                    nc.vector.tensor_mul(xrh[:], hi[:], cm[:])
                    nc.vector.tensor_add(xrh[:], xrh[:].bitcast(f32), t2[:])
                    xr[m], xr[m + 4] = xrl, xrh
                    nc.sync.dma_start(xd[m][:], lo[:])
                    nc.sync.dma_start(xd[m + 4][:], hi[:])

                # ---- C: S = xr @ xr^T streamed; aT += v^T @ S ----
                aT_ps = [psB.tile([P, T], f32, tag="psB", name=f"aTps{m}")
                         for m in range(ND)]
                # aT matmuls for tile k are emitted after the scores matmuls
                # of tile k+1, so the PE never waits on the ACT eviction.
                s_tiles = [None] * NT

                def emit_aT(k):
                    for m in range(ND):
                        for n in range(2):
                            nc.tensor.matmul(
                                aT_ps[m][:, ts(n, 512)],
                                v_sb[k][:, ts(m, P)],
                                s_tiles[k][:, ts(n, 512)],
                                start=(k == 0), stop=(k == NT - 1))

                for k in range(NT):
                    sps = psA.tile([P, T], f32, tag="psA", name="sps")
                    for n in range(2):
                        for kk in range(NDh):
                            nc.tensor.matmul(
                                sps[:, ts(n, 512)],
                                xr[kk][:, ts(k, P)],
                                xr[kk][:, ts(n, 512)],
                                start=(kk == 0), stop=(kk == NDh - 1))
                    s_sb = spool.tile([P, T], f32r, tag="score", name=f"s{k}")
                    nc.scalar.copy(s_sb[:], sps[:])
                    s_tiles[k] = s_sb
                    if k > 0:
                        emit_aT(k - 1)
                emit_aT(NT - 1)
                aT = []
                for m in range(ND):
                    at = apool.tile([P, T], f32r, tag="aT", name=f"aT{m}")
                    nc.scalar.copy(at[:], aT_ps[m][:])
                    aT.append(at)

                # ---- D/E: yT = relu(Dy^T @ aT) * xT ; zT += E_h^T @ yT ----
                # z is accumulated TRANSPOSED ([D,T]: 4 N=512 matmuls per k
                # instead of 8 N=256, and every psum group owns a full bank).
                # The z matmuls for tile k are emitted after the y matmuls of
                # tile k+1 so the PE never waits on the DVE relu*x fusion.
                z_ps = [psB.tile([P, T], f32, tag="psB", name=f"zps{i}")
                        for i in range(2)]
                y_tiles = [None] * NDh

                def emit_z(k):
                    for m in range(ND):
                        for n in range(2):
                            nc.tensor.matmul(
                                z_ps[m][:, ts(n, 512)],
                                eh_sb[8 * j + k][:, ts(m, P)],
                                y_tiles[k][:, ts(n, 512)],
                                start=(k == 0), stop=(k == NDh - 1))

                for k in range(NDh):
                    yps = psA.tile([P, T], f32, tag="psA", name="yps")
                    for n in range(2):
                        for kk in range(ND):
                            nc.tensor.matmul(
                                yps[:, ts(n, 512)],
                                dy_sb[2 * j + kk][:, ts(k, P)],
                                aT[kk][:, ts(n, 512)],
                                start=(kk == 0), stop=(kk == ND - 1))
                    xt2 = xpool.tile([P, T], f32, tag="xT", name=f"xre{k}")
                    nc.sync.dma_start(xt2[:], xd[k][:])
                    y_sb = ypool.tile([P, T], f32r, tag="yT", name=f"y{k}")
                    # y = max(yps, 0) * x   (fused relu+mul on DVE, f32r out)
                    nc.vector.scalar_tensor_tensor(
                        y_sb[:], yps[:], 0.0, xt2[:], op0=ALU.max, op1=ALU.mult)
                    y_tiles[k] = y_sb
                    if k > 0:
                        emit_z(k - 1)
                emit_z(NDh - 1)
                if j == 0:
                    for i in range(2):
                        nc.scalar.copy(z_sb[i][:], z_ps[i][:])
                else:
                    for i in range(2):
                        nc.vector.scalar_tensor_tensor(
                            z_sb[i][:], z_ps[i][:], 0.0, z_sb[i][:],
                            op0=ALU.add, op1=ALU.add)

            # ---- boundary: AllReduce(z) over the core pair, then v update ----
            zin = dpool.tile([2 * P, T], f32, tag="zin", name=f"zin{layer}")
            zout = dpool.tile([2 * P, T], f32, tag="zout", name=f"zout{layer}")
            for i in range(2):
                nc.sync.dma_start(zin[ts(i, P), :], z_sb[i][:])
            nc.gpsimd.collective_compute(
                "AllReduce", mybir.AluOpType.add,
                ins=[zin.opt()], outs=[zout.opt()], replica_groups=rg)
            # reuse the z slots for the reduced result (z is dead after the
            # DMA into zin)
            zr_sb = [zpool.tile([P, T], f32, tag=f"z{i}", name=f"zr{i}_{layer}")
                     for i in range(2)]
            for i in range(2):
                nc.sync.dma_start(zr_sb[i][:], zout[ts(i, P), :])
            # transpose zT [D,T] back to z [T,D] (2x8 PE transposes, during
            # the boundary when the PE is otherwise idle)
            # the score slots are idle across the boundary; borrow them
            zq = [spool.tile([P, T], f32, tag="score", name=f"zq{i}_{layer}")
                  for i in range(2)]
            for m in range(NT):
                for kd in range(ND):
                    tzp = psA.tile([P, P], f32, tag="psA", name="tzp")
                    nc.tensor.transpose(
                        tzp[:], zr_sb[kd][:, ts(m, P)], ident[:])
                    nc.scalar.copy(
                        zq[m // 4][:, (m % 4) * D + kd * P:
                                   (m % 4) * D + (kd + 1) * P], tzp[:])
            for m in range(NT):
                zb = zq[m // 4][:, ts(m % 4, D)]
                u = lnpool.tile([P, D], f32, tag="u", name=f"u{m}")
                layer_norm(zb, u[:])
                w = lnpool.tile([P, D], f32, tag="w", name=f"w{m}")
                nc.vector.tensor_add(w[:], v_sb[m][:].bitcast(f32), u[:])
                layer_norm(w[:], v_sb[m][:])
            transpose_v()

        # ---- readout ----
        for m in range(NT):
            rps = psA.tile([P, V], f32, tag="psA", name="rps")
            for k in range(ND):
                nc.tensor.matmul(rps[:], vT_sb[k][:, ts(m, P)], ro_sb[k][:],
                                 start=(k == 0), stop=(k == ND - 1))
            o_sb = lnpool.tile([P, V], f32, tag="o", name=f"o{m}")
            nc.scalar.copy(o_sb[:], rps[:])
            nc.sync.dma_start(d_out[ts(m, P), :], o_sb[:])

    nc.compile()
    return nc


def _get_program():
    if "nc" not in _CACHE:
        _CACHE["nc"] = _build_program()
    return _CACHE["nc"]


def _rope_tables():
    inv = (1.0 / (10000.0 ** (np.arange(0, Dh, 2, dtype=np.float32) / Dh)))
    tt = np.arange(T, dtype=np.float32)
    freqs = np.outer(tt, inv).astype(np.float32)  # [T, Dh/2]
    cosT = np.ascontiguousarray(np.cos(freqs).T, dtype=np.float32)
    sinT = np.ascontiguousarray(np.sin(freqs).T, dtype=np.float32)
    return cosT, sinT


def kernel(**inputs):
    global LAST_RESULT
    from concourse import bass_utils

    tokens = np.asarray(inputs["tokens"])
    emb_w = np.ascontiguousarray(inputs["emb_w"], dtype=np.float32)
    E = np.ascontiguousarray(inputs["E"], dtype=np.float32)
    Dx = np.ascontiguousarray(inputs["Dx"], dtype=np.float32)
    Dy = np.ascontiguousarray(inputs["Dy"], dtype=np.float32)
    readout = np.ascontiguousarray(inputs["readout"], dtype=np.float32)

    cosT, sinT = _rope_tables()

    in_maps = []
    for c in range(NCORES):
        b, hp = c // 2, c % 2
        oh = np.zeros((V, T), dtype=np.float32)
        oh[np.asarray(tokens[b], dtype=np.int64), np.arange(T)] = 1.0
        in_maps.append({
            "onehotT": oh,
            "emb_w": emb_w,
            "dx": np.ascontiguousarray(
                Dx[2 * hp:2 * hp + 2].reshape(2 * D, Dh)),
            "dy": np.ascontiguousarray(
                Dy[2 * hp:2 * hp + 2].reshape(2 * D, Dh)),
            "eh": np.ascontiguousarray(E[2 * hp * Dh:(2 * hp + 2) * Dh]),
            "cosT": cosT,
            "sinT": sinT,
            "readout": readout,
        })

    nc = _get_program()
    res = bass_utils.run_bass_kernel_spmd(
        nc, in_maps, core_ids=list(range(NCORES)),
        trace=bool(int(os.environ.get("KERNEL_TRACE", "0"))))
    LAST_RESULT = res
    out = np.stack([res.results[2 * b]["out"] for b in range(B)], axis=0)
    return out
